# revision 5
# baseline (speedup 1.0000x reference)
"""Trainium2 Bass kernel for LogWignerCrystalSlaterFixedCYJastrow.

Computes, per walker (batch of 1024, 64 electrons in 3D, box L=20):
    out = logdet(Phi_up) + logdet(Phi_dn) + jastrow
where Phi_s are 32x32 Gaussian-orbital Slater matrices over 27 periodic
images (collapsed analytically to a separable per-axis 3-image sum), and
jastrow is a Coulomb-Yukawa pair sum with minimum-image wrapping.

Strategy: pure data parallel over 8 NeuronCores, 128 walkers per core,
one walker per SBUF partition.  The two 32x32 slogdets per walker are done
with a batched, in-SBUF Gaussian elimination with partial pivoting
(pivot row selected/extracted with mask + indicator arithmetic -- no data
dependent control flow, identical instruction stream for all walkers).
"""

import os
import sys
import numpy as np
from contextlib import ExitStack

for _p in ("/opt/trn_rl_repo", "/opt/pypackages"):
    if _p not in sys.path:
        sys.path.append(_p)

import concourse.bass as bass
import concourse.bacc as bacc
import concourse.mybir as mybir
import concourse.tile as tile
from concourse.bass import AP
from concourse.bass_utils import run_bass_kernel_spmd

P = 128          # partitions = walkers per core
NCORES = 8
B = 1024
N = 64           # electrons per walker
NS = 32          # electrons / orbitals per spin
L = 20.0
F32 = mybir.dt.float32
AF = mybir.ActivationFunctionType
OP = mybir.AluOpType
AX = mybir.AxisListType


def _centers():
    n = 1
    while n ** 3 < NS:
        n += 1
    a = L / n
    coords = np.linspace(0.0, L - a, n)
    grid = np.stack(np.meshgrid(coords, coords, coords, indexing="ij"), axis=-1)
    grid = grid.reshape(-1, 3)
    cu = grid[:NS].astype(np.float32)
    cd = (grid + a / 2)[:NS].astype(np.float32)
    return cu, cd


def _jastrow_consts():
    dens = np.float32(N / L ** 3)
    A = np.float32(1.0) / np.sqrt(np.float32(4 * np.pi) * dens, dtype=np.float32)
    Fs = np.sqrt(np.float32(2.0) * A, dtype=np.float32)
    Fd = np.sqrt(A, dtype=np.float32)
    return float(A), float(Fs), float(Fd)


def _build(alpha: float) -> bass.Bass:
    nc = bacc.Bacc()
    xsh = nc.declare_dram_parameter("xsh", [P, 3, N], F32, isOutput=False)
    cst = nc.declare_dram_parameter("cst", [P, 3, 2, NS], F32, isOutput=False)
    wcs = nc.declare_dram_parameter("wcs", [P, 2, NS], F32, isOutput=False)
    outp = nc.declare_dram_parameter("out", [P, 1], F32, isOutput=True)

    aL2 = float(alpha * L * L)
    s2aL = float(2.0 * alpha * L)
    Aj, Fs, Fd = _jastrow_consts()

    with ExitStack() as ctx:
        tc = ctx.enter_context(tile.TileContext(nc))
        pool = ctx.enter_context(tc.tile_pool(name="main", bufs=1))

        # ---- loads & small constants ----
        xe = pool.tile([P, 3, N], F32, tag="xe")
        nc.default_dma_engine.dma_start(xe, xsh[:])
        ce = pool.tile([P, 3, 2, NS], F32, tag="ce")
        nc.default_dma_engine.dma_start(ce, cst[:])
        wt = pool.tile([P, 2, NS], F32, tag="wt")
        nc.default_dma_engine.dma_start(wt, wcs[:])

        biasc = pool.tile([P, 4], F32, tag="biasc")
        nc.gpsimd.memset(biasc[:, 0:1], -aL2)        # Exp image bias
        nc.gpsimd.memset(biasc[:, 1:2], -L / 2)      # Abs bias
        nc.gpsimd.memset(biasc[:, 2:3], L / 2)       # Square bias
        nc.gpsimd.memset(biasc[:, 3:4], 1e-37)       # Ln guard bias

        # =========================================================
        # Slater matrices  A[p, s, i, j] (row-major, j fastest)
        #   f_axis = e0 * (1 + p+ + p-),   Phi = fx*fy*fz
        # =========================================================
        Abuf = pool.tile([P, 2, NS, NS], F32, tag="Abuf")
        prod = pool.tile([P, 2, NS, NS], F32, tag="prod")
        dbuf = pool.tile([P, 2, NS, NS], F32, tag="dbuf")
        t2 = pool.tile([P, 2, NS, NS], F32, tag="t2")
        t3 = pool.tile([P, 2, NS, NS], F32, tag="t3")
        t4 = pool.tile([P, 2, NS, NS], F32, tag="t4")
        t5 = pool.tile([P, 2, NS, NS], F32, tag="t5")

        for c in range(3):
            xi = xe[:, c, :].rearrange("p (s i) -> p s i", s=2)
            xi = xi[:, :, :, None].broadcast_to([P, 2, NS, NS])
            cj = ce[:, c][:, :, None, :].broadcast_to([P, 2, NS, NS])
            nc.vector.tensor_tensor(dbuf, xi, cj, OP.subtract)
            nc.vector.tensor_tensor(t2, dbuf, dbuf, OP.mult)
            nc.scalar.activation(t3, t2, AF.Exp, scale=-alpha)               # e0
            nc.scalar.activation(t4, dbuf, AF.Exp,
                                 bias=biasc[:, 0:1], scale=-s2aL)            # p+
            nc.scalar.activation(t5, dbuf, AF.Exp,
                                 bias=biasc[:, 0:1], scale=s2aL)             # p-
            nc.vector.tensor_tensor(t4, t4, t5, OP.add)                      # q
            dst = prod if c == 0 else t5
            # f = (q + 1) * e0
            nc.vector.scalar_tensor_tensor(dst, t4, 1.0, t3, OP.add, OP.mult)
            if c == 1:
                nc.vector.tensor_tensor(prod, prod, t5, OP.mult)
            elif c == 2:
                nc.vector.tensor_tensor(Abuf, prod, t5, OP.mult)

        # =========================================================
        # Jastrow (gpsimd + scalar engines; DVE stays on the GE)
        # =========================================================
        jacc = pool.tile([P, N, N], F32, tag="jacc")
        jt1 = pool.tile([P, N, N], F32, tag="jt1")
        jt2 = pool.tile([P, N, N], F32, tag="jt2")
        jt3 = pool.tile([P, N, N], F32, tag="jt3")

        for c in range(3):
            xc = xe[:, c, :]
            nc.gpsimd.tensor_tensor(
                jt1,
                xc[:, :, None].broadcast_to([P, N, N]),
                xc[:, None, :].broadcast_to([P, N, N]),
                OP.subtract,
            )                                                               # dx
            nc.scalar.activation(jt2, jt1, AF.Abs)                          # u=|dx|
            nc.scalar.activation(jt1, jt2, AF.Abs, bias=biasc[:, 1:2])      # b=|u-10|
            dst = jacc if c == 0 else jt2
            # wrapped squared axis distance: (10 - b)^2
            nc.scalar.activation(dst, jt1, AF.Square,
                                 bias=biasc[:, 2:3], scale=-1.0)
            if c > 0:
                nc.gpsimd.tensor_tensor(jacc, jacc, jt2, OP.add)

        # kill the diagonal: r2 := 1000 => decay underflows to exactly 0
        diag = AP(jacc.tensor, jacc.offset, [list(jacc.ap[0]), [N + 1, N]])
        nc.gpsimd.memset(diag, 1000.0)

        nc.scalar.activation(jt1, jacc, AF.Ln)                              # ln r2
        nc.scalar.activation(jt2, jt1, AF.Exp, scale=-0.5)                  # q=1/r
        nc.gpsimd.tensor_tensor(jt1, jacc, jt2, OP.mult)                    # r
        for (a0, b0, Fv) in ((0, 0, Fs), (NS, NS, Fs), (0, NS, Fd), (NS, 0, Fd)):
            nc.scalar.activation(
                jt3[:, a0:a0 + NS, b0:b0 + NS],
                jt1[:, a0:a0 + NS, b0:b0 + NS],
                AF.Exp,
                scale=-1.0 / Fv,
            )                                                               # e
        # x2 = min(r2/100, (1-1e-5)^2)   (in place over jacc)
        nc.gpsimd.tensor_scalar(
            jacc, jacc, 0.01, float((1 - 1e-5) ** 2), OP.mult, OP.min
        )
        nc.gpsimd.tensor_scalar(jt1, jacc, -1.0, 1.0, OP.mult, OP.add)      # w=1-x2
        nc.scalar.activation(jacc, jt1, AF.Ln)                              # ln w
        nc.scalar.activation(jt1, jacc, AF.Exp, scale=-1.0)                 # 1/w
        nc.scalar.activation(jacc, jt1, AF.Exp, bias=1.0, scale=-1.0)       # decay
        nc.gpsimd.tensor_scalar(jt3, jt3, -1.0, 1.0, OP.mult, OP.add)       # 1-e
        nc.gpsimd.tensor_tensor(jt3, jt3, jacc, OP.mult)                    # *decay
        nc.gpsimd.tensor_tensor(jt3, jt3, jt2, OP.mult)                     # *1/r
        jsum = pool.tile([P, 1], F32, tag="jsum")
        nc.scalar.activation(jt1, jt3, AF.Copy, scale=-0.5 * Aj,
                             accum_out=jsum)

        # =========================================================
        # Batched Gaussian elimination w/ partial pivoting (both spins)
        # =========================================================
        maskb = pool.tile([P, 2, NS], F32, tag="maskb")
        cbuf = pool.tile([P, 2, NS], F32, tag="cbuf")
        indb = pool.tile([P, 2, NS], F32, tag="indb")
        Mb = pool.tile([P, 2], F32, tag="Mb")
        mb = pool.tile([P, 2, NS], F32, tag="mb")
        rpv = pool.tile([P, 2, 1], F32, tag="rpv")
        prowall = pool.tile([P, 2, NS, NS], F32, tag="prowall")
        scr = pool.tile([P, 2, NS, NS], F32, tag="scr")
        nc.gpsimd.memset(maskb, 1.0)

        A_ji = Abuf.rearrange("p s i j -> p s j i")  # (P, 2, j, i)

        for k in range(NS):
            T = NS - k
            colk = Abuf[:, :, :, k]                                  # (P,2,NS)
            nc.scalar.activation(cbuf, colk, AF.Abs)
            nc.vector.tensor_tensor(cbuf, cbuf, maskb, OP.mult)
            nc.vector.tensor_tensor(cbuf, cbuf, wt, OP.mult)
            nc.vector.reduce_max(Mb, cbuf, axis=AX.X)
            nc.vector.tensor_tensor(
                indb, cbuf, Mb[:, :, None].broadcast_to([P, 2, NS]), OP.is_equal
            )
            # pivot row extraction: prow[j'] = sum_i ind[i] * A[i, k+j']
            nc.vector.tensor_tensor(
                scr[:, :, :T, :],
                A_ji[:, :, k:, :],
                indb[:, :, None, :].broadcast_to([P, 2, T, NS]),
                OP.mult,
            )
            nc.vector.reduce_sum(prowall[:, :, k, :T], scr[:, :, :T, :], axis=AX.X)
            if k == NS - 1:
                break
            nc.vector.tensor_tensor(maskb, maskb, indb, OP.is_gt)
            nc.vector.reciprocal(rpv, prowall[:, :, k, 0:1])
            nc.vector.tensor_tensor(
                mb, colk, rpv.broadcast_to([P, 2, NS]), OP.mult
            )
            nc.vector.tensor_tensor(mb, mb, maskb, OP.mult)
            nc.vector.tensor_tensor(
                scr[:, :, :, :T],
                mb[:, :, :, None].broadcast_to([P, 2, NS, T]),
                prowall[:, :, k, None, :T].broadcast_to([P, 2, NS, T]),
                OP.mult,
            )
            nc.vector.tensor_tensor(
                Abuf[:, :, :, k:], Abuf[:, :, :, k:], scr[:, :, :, :T], OP.subtract
            )

        # logdet = sum_k log|pval_k| per spin, then sum spins + jastrow
        labs = pool.tile([P, 2, NS], F32, tag="labs")
        lgb = pool.tile([P, 2, NS], F32, tag="lgb")
        nc.scalar.activation(labs, prowall[:, :, :, 0], AF.Abs)
        nc.scalar.activation(lgb, labs, AF.Ln, bias=biasc[:, 3:4])
        ld2 = pool.tile([P, 2], F32, tag="ld2")
        nc.vector.reduce_sum(ld2, lgb, axis=AX.X)
        ld1 = pool.tile([P, 1], F32, tag="ld1")
        nc.vector.reduce_sum(ld1, ld2, axis=AX.X)
        ob = pool.tile([P, 1], F32, tag="ob")
        nc.vector.tensor_tensor(ob, ld1, jsum, OP.add)
        nc.default_dma_engine.dma_start(outp[:], ob)

    nc.finalize()
    return nc


_CACHE = {}


def _get_built(alpha: float):
    key = round(alpha, 9)
    if key not in _CACHE:
        _CACHE[key] = _build(alpha)
    return _CACHE[key]


def _make_inputs(walkerRs: np.ndarray):
    cu, cd = _centers()
    cen = np.stack([cu, cd], 0)                   # (2, NS, 3)
    cst = np.ascontiguousarray(
        np.broadcast_to(cen.transpose(2, 0, 1)[None], (P, 3, 2, NS))
    ).astype(np.float32)
    w = (1.0 + np.arange(NS) * 2.0 ** -21).astype(np.float32)
    wcs = np.ascontiguousarray(
        np.broadcast_to(w[None, None, :], (P, 2, NS))
    ).astype(np.float32)
    in_maps = []
    for c in range(NCORES):
        sh = walkerRs[c * P:(c + 1) * P]          # (P, N, 3)
        xsh = np.ascontiguousarray(sh.transpose(0, 2, 1)).astype(np.float32)
        in_maps.append({"xsh": xsh, "cst": cst, "wcs": wcs})
    return in_maps


def kernel(walkerRs: np.ndarray, log_alpha: np.ndarray, _trace=False):
    walkerRs = np.asarray(walkerRs, dtype=np.float32)
    la = float(np.asarray(log_alpha))
    alpha = float(np.clip(np.exp(la), 55.0 / L ** 2, 300.0 / L ** 2))
    nc = _get_built(alpha)
    in_maps = _make_inputs(walkerRs)
    res = run_bass_kernel_spmd(nc, in_maps, list(range(NCORES)), trace=_trace)
    out = np.concatenate([res.results[i]["out"][:, 0] for i in range(NCORES)])
    if _trace:
        return out.astype(np.float32), res
    return out.astype(np.float32)


# revision 7
# speedup vs baseline: 1.0001x; 1.0001x over previous
"""Trainium2 Bass kernel for LogWignerCrystalSlaterFixedCYJastrow.

Computes, per walker (batch of 1024, 64 electrons in 3D, box L=20):
    out = logdet(Phi_up) + logdet(Phi_dn) + jastrow
where Phi_s are 32x32 Gaussian-orbital Slater matrices over 27 periodic
images (collapsed analytically to a separable per-axis 3-image sum), and
jastrow is a Coulomb-Yukawa pair sum with minimum-image wrapping.

Strategy: pure data parallel over 8 NeuronCores, 128 walkers per core,
one walker per SBUF partition.  The two 32x32 slogdets per walker are done
with a batched, in-SBUF Gaussian elimination with partial pivoting
(pivot row selected/extracted with mask + indicator arithmetic -- no data
dependent control flow, identical instruction stream for all walkers).
"""

import os
import sys
import numpy as np
from contextlib import ExitStack

for _p in ("/opt/trn_rl_repo", "/opt/pypackages"):
    if _p not in sys.path:
        sys.path.append(_p)

import concourse.bass as bass
import concourse.bacc as bacc
import concourse.mybir as mybir
import concourse.tile as tile
from concourse.bass import AP
from concourse.bass_utils import run_bass_kernel_spmd

P = 128          # partitions = walkers per core
NCORES = 8
B = 1024
N = 64           # electrons per walker
NS = 32          # electrons / orbitals per spin
L = 20.0
F32 = mybir.dt.float32
AF = mybir.ActivationFunctionType
OP = mybir.AluOpType
AX = mybir.AxisListType


def _centers():
    n = 1
    while n ** 3 < NS:
        n += 1
    a = L / n
    coords = np.linspace(0.0, L - a, n)
    grid = np.stack(np.meshgrid(coords, coords, coords, indexing="ij"), axis=-1)
    grid = grid.reshape(-1, 3)
    cu = grid[:NS].astype(np.float32)
    cd = (grid + a / 2)[:NS].astype(np.float32)
    return cu, cd


def _jastrow_consts():
    dens = np.float32(N / L ** 3)
    A = np.float32(1.0) / np.sqrt(np.float32(4 * np.pi) * dens, dtype=np.float32)
    Fs = np.sqrt(np.float32(2.0) * A, dtype=np.float32)
    Fd = np.sqrt(A, dtype=np.float32)
    return float(A), float(Fs), float(Fd)


def _build(alpha: float) -> bass.Bass:
    nc = bacc.Bacc()
    xsh = nc.declare_dram_parameter("xsh", [P, 3, N], F32, isOutput=False)
    cst = nc.declare_dram_parameter("cst", [P, 3, 2, NS], F32, isOutput=False)
    wcs = nc.declare_dram_parameter("wcs", [P, 2, NS], F32, isOutput=False)
    outp = nc.declare_dram_parameter("out", [P, 1], F32, isOutput=True)

    aL2 = float(alpha * L * L)
    s2aL = float(2.0 * alpha * L)
    Aj, Fs, Fd = _jastrow_consts()

    with ExitStack() as ctx:
        tc = ctx.enter_context(tile.TileContext(nc))
        pool = ctx.enter_context(tc.tile_pool(name="main", bufs=1))

        # ---- loads & small constants ----
        xe = pool.tile([P, 3, N], F32, tag="xe")
        nc.default_dma_engine.dma_start(xe, xsh[:])
        ce = pool.tile([P, 3, 2, NS], F32, tag="ce")
        nc.default_dma_engine.dma_start(ce, cst[:])
        biasc = pool.tile([P, 4], F32, tag="biasc")
        nc.gpsimd.memset(biasc[:, 0:1], -aL2)        # Exp image bias
        nc.gpsimd.memset(biasc[:, 1:2], -L / 2)      # Abs bias
        nc.gpsimd.memset(biasc[:, 2:3], L / 2)       # Square bias
        nc.gpsimd.memset(biasc[:, 3:4], 1e-37)       # Ln guard bias

        # =========================================================
        # Slater matrices  A[p, s, i, j] (row-major, j fastest)
        #   f_axis = e0 * (1 + p+ + p-),   Phi = fx*fy*fz
        # =========================================================
        Abuf = pool.tile([P, 2, NS, NS], F32, tag="Abuf")
        prod = pool.tile([P, 2, NS, NS], F32, tag="prod")
        dbuf = pool.tile([P, 2, NS, NS], F32, tag="dbuf")
        t2 = pool.tile([P, 2, NS, NS], F32, tag="t2")
        t3 = pool.tile([P, 2, NS, NS], F32, tag="t3")
        t4 = pool.tile([P, 2, NS, NS], F32, tag="t4")
        t5 = pool.tile([P, 2, NS, NS], F32, tag="t5")

        for c in range(3):
            # column-major: d[p, s, j, i] = x_i - c_j
            xi = xe[:, c, :].rearrange("p (s i) -> p s i", s=2)
            xi = xi[:, :, None, :].broadcast_to([P, 2, NS, NS])
            cj = ce[:, c][:, :, :, None].broadcast_to([P, 2, NS, NS])
            nc.vector.tensor_tensor(dbuf, xi, cj, OP.subtract)
            nc.gpsimd.tensor_tensor(t2, dbuf, dbuf, OP.mult)
            nc.scalar.activation(t3, t2, AF.Exp, scale=-alpha)               # e0
            nc.scalar.activation(t4, dbuf, AF.Exp,
                                 bias=biasc[:, 0:1], scale=-s2aL)            # p+
            nc.scalar.activation(t5, dbuf, AF.Exp,
                                 bias=biasc[:, 0:1], scale=s2aL)             # p-
            nc.gpsimd.tensor_tensor(t4, t4, t5, OP.add)                      # q
            dst = prod if c == 0 else t5
            # f = (q + 1) * e0
            nc.gpsimd.tensor_scalar_add(t4, t4, 1.0)
            nc.gpsimd.tensor_tensor(dst, t4, t3, OP.mult)
            if c == 1:
                nc.vector.tensor_tensor(prod, prod, t5, OP.mult)
            elif c == 2:
                nc.vector.tensor_tensor(Abuf, prod, t5, OP.mult)

        # =========================================================
        # Jastrow (gpsimd + scalar engines; DVE stays on the GE)
        # =========================================================
        jacc = pool.tile([P, N, N], F32, tag="jacc")
        jt1 = pool.tile([P, N, N], F32, tag="jt1")
        jt2 = pool.tile([P, N, N], F32, tag="jt2")
        jt3 = pool.tile([P, N, N], F32, tag="jt3")

        for c in range(3):
            xc = xe[:, c, :]
            nc.gpsimd.tensor_tensor(
                jt1,
                xc[:, :, None].broadcast_to([P, N, N]),
                xc[:, None, :].broadcast_to([P, N, N]),
                OP.subtract,
            )                                                               # dx
            nc.scalar.activation(jt2, jt1, AF.Abs)                          # u=|dx|
            nc.scalar.activation(jt1, jt2, AF.Abs, bias=biasc[:, 1:2])      # b=|u-10|
            dst = jacc if c == 0 else jt2
            # wrapped squared axis distance: (10 - b)^2
            nc.scalar.activation(dst, jt1, AF.Square,
                                 bias=biasc[:, 2:3], scale=-1.0)
            if c > 0:
                nc.gpsimd.tensor_tensor(jacc, jacc, jt2, OP.add)

        # kill the diagonal: r2 := 1000 => decay underflows to exactly 0
        diag = AP(jacc.tensor, jacc.offset, [list(jacc.ap[0]), [N + 1, N]])
        nc.gpsimd.memset(diag, 1000.0)

        nc.scalar.activation(jt1, jacc, AF.Ln)                              # ln r2
        nc.scalar.activation(jt2, jt1, AF.Exp, scale=-0.5)                  # q=1/r
        nc.gpsimd.tensor_tensor(jt1, jacc, jt2, OP.mult)                    # r
        for (a0, b0, Fv) in ((0, 0, Fs), (NS, NS, Fs), (0, NS, Fd), (NS, 0, Fd)):
            nc.scalar.activation(
                jt3[:, a0:a0 + NS, b0:b0 + NS],
                jt1[:, a0:a0 + NS, b0:b0 + NS],
                AF.Exp,
                scale=-1.0 / Fv,
            )                                                               # e
        # x2 = min(r2/100, (1-1e-5)^2)   (in place over jacc)
        nc.vector.tensor_scalar(
            jacc, jacc, 0.01, float((1 - 1e-5) ** 2), OP.mult, OP.min
        )
        nc.gpsimd.tensor_scalar(jt1, jacc, -1.0, 1.0, OP.mult, OP.add)      # w=1-x2
        nc.scalar.activation(jacc, jt1, AF.Ln)                              # ln w
        nc.scalar.activation(jt1, jacc, AF.Exp, scale=-1.0)                 # 1/w
        nc.scalar.activation(jacc, jt1, AF.Exp, bias=1.0, scale=-1.0)       # decay
        nc.gpsimd.tensor_scalar(jt3, jt3, -1.0, 1.0, OP.mult, OP.add)       # 1-e
        nc.gpsimd.tensor_tensor(jt3, jt3, jacc, OP.mult)                    # *decay
        nc.gpsimd.tensor_tensor(jt3, jt3, jt2, OP.mult)                     # *1/r
        jsum = pool.tile([P, 1], F32, tag="jsum")
        nc.scalar.activation(jt1, jt3, AF.Copy, scale=-0.5 * Aj,
                             accum_out=jsum)

        # =========================================================
        # Batched Gaussian elimination w/ partial pivoting (both spins)
        # =========================================================
        maskw = pool.tile([P, 2, NS], F32, tag="maskw")
        c2b = pool.tile([P, 2, NS], F32, tag="c2b")
        c2m = pool.tile([P, 2, NS], F32, tag="c2m")
        indb = pool.tile([P, 2, NS], F32, tag="indb")
        Mb = pool.tile([P, 2], F32, tag="Mb")
        rpv = pool.tile([P, 2, 1], F32, tag="rpv")
        prwb = pool.tile([P, 2, NS], F32, tag="prwb")
        prowall = pool.tile([P, 2, NS, NS], F32, tag="prowall")
        scr = pool.tile([P, 2, NS, NS], F32, tag="scr")
        # maskw holds the tie-break weights; used rows go negative (-2 trick)
        nc.default_dma_engine.dma_start(maskw, wcs[:])

        for k in range(NS):
            T = NS - k
            colk = Abuf[:, :, k, :]                                  # (P,2,NS) contiguous
            nc.vector.tensor_tensor(c2b, colk, colk, OP.mult)
            nc.vector.tensor_tensor(c2m, c2b, maskw, OP.mult)
            nc.vector.reduce_max(Mb, c2m, axis=AX.X)
            nc.vector.tensor_tensor(
                indb, c2m, Mb[:, :, None].broadcast_to([P, 2, NS]), OP.is_equal
            )
            # pivot row extraction: prow[j'] = sum_i ind[i] * A[j'=k.., i]
            nc.vector.tensor_tensor(
                scr[:, :, :T, :],
                Abuf[:, :, k:, :],
                indb[:, :, None, :].broadcast_to([P, 2, T, NS]),
                OP.mult,
            )
            nc.vector.reduce_sum(prowall[:, :, k, :T], scr[:, :, :T, :], axis=AX.X)
            if k == NS - 1:
                break
            nc.vector.scalar_tensor_tensor(
                maskw, indb, -2.0, maskw, OP.mult, OP.add
            )
            nc.vector.reciprocal(rpv, prowall[:, :, k, 0:1])
            nc.vector.tensor_tensor(
                prwb[:, :, :T - 1], prowall[:, :, k, 1:T],
                rpv.broadcast_to([P, 2, T - 1]), OP.mult,
            )
            # update columns k+1.. for ALL rows: pivot row self-annihilates,
            # used rows are dead storage (negative maskw keeps them unpicked)
            nc.vector.tensor_tensor(
                scr[:, :, :T - 1, :],
                colk[:, :, None, :].broadcast_to([P, 2, T - 1, NS]),
                prwb[:, :, :T - 1, None].broadcast_to([P, 2, T - 1, NS]),
                OP.mult,
            )
            nc.vector.tensor_tensor(
                Abuf[:, :, k + 1:, :], Abuf[:, :, k + 1:, :],
                scr[:, :, :T - 1, :], OP.subtract
            )

        # logdet = sum_k log|pval_k| per spin, then sum spins + jastrow
        labs = pool.tile([P, 2, NS], F32, tag="labs")
        lgb = pool.tile([P, 2, NS], F32, tag="lgb")
        nc.scalar.activation(labs, prowall[:, :, :, 0], AF.Abs)
        nc.scalar.activation(lgb, labs, AF.Ln, bias=biasc[:, 3:4])
        ld2 = pool.tile([P, 2], F32, tag="ld2")
        nc.vector.reduce_sum(ld2, lgb, axis=AX.X)
        ld1 = pool.tile([P, 1], F32, tag="ld1")
        nc.vector.reduce_sum(ld1, ld2, axis=AX.X)
        ob = pool.tile([P, 1], F32, tag="ob")
        nc.vector.tensor_tensor(ob, ld1, jsum, OP.add)
        nc.default_dma_engine.dma_start(outp[:], ob)

    nc.finalize()
    return nc


_CACHE = {}


def _get_built(alpha: float):
    key = round(alpha, 9)
    if key not in _CACHE:
        _CACHE[key] = _build(alpha)
    return _CACHE[key]


def _make_inputs(walkerRs: np.ndarray):
    cu, cd = _centers()
    cen = np.stack([cu, cd], 0)                   # (2, NS, 3)
    cst = np.ascontiguousarray(
        np.broadcast_to(cen.transpose(2, 0, 1)[None], (P, 3, 2, NS))
    ).astype(np.float32)
    w = (1.0 + np.arange(NS) * 2.0 ** -21).astype(np.float32)
    wcs = np.ascontiguousarray(
        np.broadcast_to(w[None, None, :], (P, 2, NS))
    ).astype(np.float32)
    in_maps = []
    for c in range(NCORES):
        sh = walkerRs[c * P:(c + 1) * P]          # (P, N, 3)
        xsh = np.ascontiguousarray(sh.transpose(0, 2, 1)).astype(np.float32)
        in_maps.append({"xsh": xsh, "cst": cst, "wcs": wcs})
    return in_maps


def kernel(walkerRs: np.ndarray, log_alpha: np.ndarray, _trace=False):
    walkerRs = np.asarray(walkerRs, dtype=np.float32)
    la = float(np.asarray(log_alpha))
    alpha = float(np.clip(np.exp(la), 55.0 / L ** 2, 300.0 / L ** 2))
    nc = _get_built(alpha)
    in_maps = _make_inputs(walkerRs)
    res = run_bass_kernel_spmd(nc, in_maps, list(range(NCORES)), trace=_trace)
    out = np.concatenate([res.results[i]["out"][:, 0] for i in range(NCORES)])
    if _trace:
        return out.astype(np.float32), res
    return out.astype(np.float32)


# revision 10
# speedup vs baseline: 1.4256x; 1.4254x over previous
"""Trainium2 Bass kernel for LogWignerCrystalSlaterFixedCYJastrow.

Computes, per walker (batch of 1024, 64 electrons in 3D, box L=20):
    out = logdet(Phi_up) + logdet(Phi_dn) + jastrow
where Phi_s are 32x32 Gaussian-orbital Slater matrices over 27 periodic
images (collapsed analytically to a separable per-axis 3-image sum), and
jastrow is a Coulomb-Yukawa pair sum with minimum-image wrapping.

Strategy: pure data parallel over 8 NeuronCores, 128 walkers per core,
one walker per SBUF partition.  The two 32x32 slogdets per walker are done
with a batched, in-SBUF Gaussian elimination with partial pivoting
(pivot row selected/extracted with indicator arithmetic -- no data
dependent control flow, identical instruction stream for all walkers).
Engine split: DVE does all 2-input elementwise + reductions, ScalarE all
transcendentals/1-input chains (they overlap cleanly), GpSimd only tiny
memsets (its big ops starve DVE through SBUF port contention).
"""

import os
import sys
import numpy as np
from contextlib import ExitStack

for _p in ("/opt/trn_rl_repo", "/opt/pypackages"):
    if _p not in sys.path:
        sys.path.append(_p)

import concourse.bass as bass
import concourse.bacc as bacc
import concourse.mybir as mybir
import concourse.tile as tile
from concourse.bass import AP
from concourse.bass_utils import run_bass_kernel_spmd

P = 128          # partitions = walkers per core
NCORES = 8
B = 1024
N = 64           # electrons per walker
NS = 32          # electrons / orbitals per spin
L = 20.0
F32 = mybir.dt.float32
AF = mybir.ActivationFunctionType
OP = mybir.AluOpType
AX = mybir.AxisListType


def _centers():
    n = 1
    while n ** 3 < NS:
        n += 1
    a = L / n
    coords = np.linspace(0.0, L - a, n)
    grid = np.stack(np.meshgrid(coords, coords, coords, indexing="ij"), axis=-1)
    grid = grid.reshape(-1, 3)
    cu = grid[:NS].astype(np.float32)
    cd = (grid + a / 2)[:NS].astype(np.float32)
    return cu, cd


def _jastrow_consts():
    dens = np.float32(N / L ** 3)
    A = np.float32(1.0) / np.sqrt(np.float32(4 * np.pi) * dens, dtype=np.float32)
    Fs = np.sqrt(np.float32(2.0) * A, dtype=np.float32)
    Fd = np.sqrt(A, dtype=np.float32)
    return float(A), float(Fs), float(Fd)


def _build(alpha: float) -> bass.Bass:
    nc = bacc.Bacc()
    xsh = nc.declare_dram_parameter("xsh", [P, 3, N], F32, isOutput=False)
    cst = nc.declare_dram_parameter("cst", [P, 3, 2, NS], F32, isOutput=False)
    wcs = nc.declare_dram_parameter("wcs", [P, 2, NS], F32, isOutput=False)
    outp = nc.declare_dram_parameter("out", [P, 1], F32, isOutput=True)

    aL2 = float(alpha * L * L)
    s2aL = float(2.0 * alpha * L)
    Aj, Fs, Fd = _jastrow_consts()
    WMIN = float(1.0 - (1.0 - 1e-5) ** 2)   # lower clamp of w = 1 - x^2

    with ExitStack() as ctx:
        tc = ctx.enter_context(tile.TileContext(nc))
        pool = ctx.enter_context(tc.tile_pool(name="main", bufs=1))

        # ---- loads & small constants ----
        xe = pool.tile([P, 3, N], F32, tag="xe")
        nc.default_dma_engine.dma_start(xe, xsh[:])
        ce = pool.tile([P, 3, 2, NS], F32, tag="ce")
        nc.default_dma_engine.dma_start(ce, cst[:])

        biasc = pool.tile([P, 4], F32, tag="biasc")
        nc.gpsimd.memset(biasc[:, 0:1], -aL2)        # Exp image bias
        nc.gpsimd.memset(biasc[:, 1:2], -L / 2)      # Abs bias
        nc.gpsimd.memset(biasc[:, 2:3], L / 2)       # Square bias
        nc.gpsimd.memset(biasc[:, 3:4], 1e-37)       # Ln guard bias

        # =========================================================
        # Slater matrices, column-major: A[p, s, j, i] = Phi[i, j]
        #   f_axis = e0 * (1 + p+ + p-),   Phi = fx*fy*fz
        # =========================================================
        Abuf = pool.tile([P, 2, NS, NS], F32, tag="Abuf")
        prod = pool.tile([P, 2, NS, NS], F32, tag="prod")
        dbuf = pool.tile([P, 2, NS, NS], F32, tag="dbuf")
        t2 = pool.tile([P, 2, NS, NS], F32, tag="t2")
        t3 = pool.tile([P, 2, NS, NS], F32, tag="t3")
        t4 = pool.tile([P, 2, NS, NS], F32, tag="t4")
        t5 = pool.tile([P, 2, NS, NS], F32, tag="t5")

        for c in range(3):
            # d[p, s, j, i] = x_i - c_j
            xi = xe[:, c, :].rearrange("p (s i) -> p s i", s=2)
            xi = xi[:, :, None, :].broadcast_to([P, 2, NS, NS])
            cj = ce[:, c][:, :, :, None].broadcast_to([P, 2, NS, NS])
            nc.vector.tensor_tensor(dbuf, xi, cj, OP.subtract)
            nc.scalar.activation(t2, dbuf, AF.Square)                        # d^2
            nc.scalar.activation(t3, t2, AF.Exp, scale=-alpha)               # e0
            nc.scalar.activation(t4, dbuf, AF.Exp,
                                 bias=biasc[:, 0:1], scale=-s2aL)            # p+
            nc.scalar.activation(t5, dbuf, AF.Exp,
                                 bias=biasc[:, 0:1], scale=s2aL)             # p-
            nc.vector.tensor_tensor(t4, t4, t5, OP.add)                      # q
            dst = prod if c == 0 else t5
            # f = (q + 1) * e0
            nc.vector.scalar_tensor_tensor(dst, t4, 1.0, t3, OP.add, OP.mult)
            if c == 1:
                nc.vector.tensor_tensor(prod, prod, t5, OP.mult)
            elif c == 2:
                nc.vector.tensor_tensor(Abuf, prod, t5, OP.mult)

        # =========================================================
        # Jastrow front: pairwise wrapped r^2 (DVE dx + ScalarE chain)
        # =========================================================
        jacc = pool.tile([P, N, N], F32, tag="jacc")
        jt1 = pool.tile([P, N, N], F32, tag="jt1")
        jt2 = pool.tile([P, N, N], F32, tag="jt2")
        jt3 = pool.tile([P, N, N], F32, tag="jt3")
        jt4 = pool.tile([P, N, N], F32, tag="jt4")

        for c in range(3):
            xc = xe[:, c, :]
            nc.vector.tensor_tensor(
                jt1,
                xc[:, :, None].broadcast_to([P, N, N]),
                xc[:, None, :].broadcast_to([P, N, N]),
                OP.subtract,
            )                                                               # dx
            nc.scalar.activation(jt2, jt1, AF.Abs)                          # u=|dx|
            nc.scalar.activation(jt1, jt2, AF.Abs, bias=biasc[:, 1:2])      # b=|u-10|
            dst = jacc if c == 0 else jt2
            # wrapped squared axis distance: (10 - b)^2
            nc.scalar.activation(dst, jt1, AF.Square,
                                 bias=biasc[:, 2:3], scale=-1.0)
            if c > 0:
                nc.vector.tensor_tensor(jacc, jacc, jt2, OP.add)

        # kill the diagonal: r2 := 1000 => decay underflows to exactly 0
        diag = AP(jacc.tensor, jacc.offset, [list(jacc.ap[0]), [N + 1, N]])
        nc.gpsimd.memset(diag, 1000.0)

        # w = max(1 - r2/100, WMIN)  (= 1 - x^2 with the reference clip)
        nc.vector.tensor_scalar(jt4, jacc, -0.01, 1.0, OP.mult, OP.add)
        nc.vector.tensor_scalar(jt4, jt4, WMIN, None, OP.max)

        # ScalarE-only chains (overlap the GE below):
        nc.scalar.activation(jt1, jacc, AF.Ln)                              # ln r2
        nc.scalar.activation(jt2, jt1, AF.Exp, scale=-0.5)                  # q=1/r
        nc.scalar.activation(jacc, jt1, AF.Exp, scale=0.5)                  # r
        for (a0, b0, Fv) in ((0, 0, Fs), (NS, NS, Fs), (0, NS, Fd), (NS, 0, Fd)):
            nc.scalar.activation(
                jt3[:, a0:a0 + NS, b0:b0 + NS],
                jacc[:, a0:a0 + NS, b0:b0 + NS],
                AF.Exp,
                scale=-1.0 / Fv,
            )                                                               # e
        nc.scalar.activation(jt1, jt4, AF.Ln)                               # ln w
        nc.scalar.activation(jt4, jt1, AF.Exp, scale=-1.0)                  # 1/w
        nc.scalar.activation(jacc, jt4, AF.Exp, bias=1.0, scale=-1.0)       # decay

        # =========================================================
        # Batched Gaussian elimination w/ partial pivoting (both spins)
        # column-major A; pivot search on squared candidates
        # =========================================================
        maskw = pool.tile([P, 2, NS], F32, tag="maskw")
        c2b = pool.tile([P, 2, NS], F32, tag="c2b")
        c2m = pool.tile([P, 2, NS], F32, tag="c2m")
        indb = pool.tile([P, 2, NS], F32, tag="indb")
        Mb = pool.tile([P, 2], F32, tag="Mb")
        rpv = pool.tile([P, 2, 1], F32, tag="rpv")
        prwb = pool.tile([P, 2, NS], F32, tag="prwb")
        prowall = pool.tile([P, 2, NS, NS], F32, tag="prowall")
        scr = pool.tile([P, 2, NS, NS], F32, tag="scr")
        # maskw holds the tie-break weights; used rows go negative (-2 trick)
        nc.default_dma_engine.dma_start(maskw, wcs[:])

        for k in range(NS):
            T = NS - k
            colk = Abuf[:, :, k, :]                                  # contiguous
            nc.vector.tensor_tensor(c2b, colk, colk, OP.mult)
            nc.vector.tensor_tensor(c2m, c2b, maskw, OP.mult)
            nc.vector.reduce_max(Mb, c2m, axis=AX.X)
            nc.vector.tensor_tensor(
                indb, c2m, Mb[:, :, None].broadcast_to([P, 2, NS]), OP.is_equal
            )
            # pivot row extraction: prow[j'] = sum_i ind[i] * A[j'=k.., i]
            nc.vector.tensor_tensor(
                scr[:, :, :T, :],
                Abuf[:, :, k:, :],
                indb[:, :, None, :].broadcast_to([P, 2, T, NS]),
                OP.mult,
            )
            nc.vector.reduce_sum(prowall[:, :, k, :T], scr[:, :, :T, :], axis=AX.X)
            if k == NS - 1:
                break
            nc.vector.scalar_tensor_tensor(
                maskw, indb, -2.0, maskw, OP.mult, OP.add
            )
            nc.vector.reciprocal(rpv, prowall[:, :, k, 0:1])
            nc.vector.tensor_tensor(
                prwb[:, :, :T - 1], prowall[:, :, k, 1:T],
                rpv.broadcast_to([P, 2, T - 1]), OP.mult,
            )
            # update columns k+1.. for ALL rows: pivot row self-annihilates,
            # used rows are dead storage (negative maskw keeps them unpicked)
            nc.vector.tensor_tensor(
                scr[:, :, :T - 1, :],
                colk[:, :, None, :].broadcast_to([P, 2, T - 1, NS]),
                prwb[:, :, :T - 1, None].broadcast_to([P, 2, T - 1, NS]),
                OP.mult,
            )
            nc.vector.tensor_tensor(
                Abuf[:, :, k + 1:, :], Abuf[:, :, k + 1:, :],
                scr[:, :, :T - 1, :], OP.subtract
            )

        # =========================================================
        # Jastrow tail + logdet reduction + combine
        # =========================================================
        nc.vector.tensor_scalar(jt1, jt3, -1.0, 1.0, OP.mult, OP.add)       # 1-e
        nc.vector.tensor_tensor(jt1, jt1, jacc, OP.mult)                    # *decay
        nc.vector.tensor_tensor(jt1, jt1, jt2, OP.mult)                     # *(1/r)
        jsum = pool.tile([P, 1], F32, tag="jsum")
        nc.scalar.activation(jt3, jt1, AF.Copy, scale=-0.5 * Aj,
                             accum_out=jsum)

        labs = pool.tile([P, 2, NS], F32, tag="labs")
        lgb = pool.tile([P, 2, NS], F32, tag="lgb")
        nc.scalar.activation(labs, prowall[:, :, :, 0], AF.Abs)
        nc.scalar.activation(lgb, labs, AF.Ln, bias=biasc[:, 3:4])
        ld2 = pool.tile([P, 2], F32, tag="ld2")
        nc.vector.reduce_sum(ld2, lgb, axis=AX.X)
        ld1 = pool.tile([P, 1], F32, tag="ld1")
        nc.vector.reduce_sum(ld1, ld2, axis=AX.X)
        ob = pool.tile([P, 1], F32, tag="ob")
        nc.vector.tensor_tensor(ob, ld1, jsum, OP.add)
        nc.default_dma_engine.dma_start(outp[:], ob)

    nc.finalize()
    return nc


_CACHE = {}


def _get_built(alpha: float):
    key = round(alpha, 9)
    if key not in _CACHE:
        _CACHE[key] = _build(alpha)
    return _CACHE[key]


def _make_inputs(walkerRs: np.ndarray):
    cu, cd = _centers()
    cen = np.stack([cu, cd], 0)                   # (2, NS, 3)
    cst = np.ascontiguousarray(
        np.broadcast_to(cen.transpose(2, 0, 1)[None], (P, 3, 2, NS))
    ).astype(np.float32)
    w = (1.0 + np.arange(NS) * 2.0 ** -21).astype(np.float32)
    wcs = np.ascontiguousarray(
        np.broadcast_to(w[None, None, :], (P, 2, NS))
    ).astype(np.float32)
    in_maps = []
    for c in range(NCORES):
        sh = walkerRs[c * P:(c + 1) * P]          # (P, N, 3)
        xsh = np.ascontiguousarray(sh.transpose(0, 2, 1)).astype(np.float32)
        in_maps.append({"xsh": xsh, "cst": cst, "wcs": wcs})
    return in_maps


def kernel(walkerRs: np.ndarray, log_alpha: np.ndarray, _trace=False):
    walkerRs = np.asarray(walkerRs, dtype=np.float32)
    la = float(np.asarray(log_alpha))
    alpha = float(np.clip(np.exp(la), 55.0 / L ** 2, 300.0 / L ** 2))
    nc = _get_built(alpha)
    in_maps = _make_inputs(walkerRs)
    res = run_bass_kernel_spmd(nc, in_maps, list(range(NCORES)), trace=_trace)
    out = np.concatenate([res.results[i]["out"][:, 0] for i in range(NCORES)])
    if _trace:
        return out.astype(np.float32), res
    return out.astype(np.float32)


# revision 11
# speedup vs baseline: 1.4564x; 1.0216x over previous
"""Trainium2 Bass kernel for LogWignerCrystalSlaterFixedCYJastrow.

Computes, per walker (batch of 1024, 64 electrons in 3D, box L=20):
    out = logdet(Phi_up) + logdet(Phi_dn) + jastrow
where Phi_s are 32x32 Gaussian-orbital Slater matrices over 27 periodic
images (collapsed analytically to a separable per-axis 3-image sum), and
jastrow is a Coulomb-Yukawa pair sum with minimum-image wrapping.

Strategy: pure data parallel over 8 NeuronCores, 128 walkers per core,
one walker per SBUF partition.  The two 32x32 slogdets per walker are done
with a batched, in-SBUF Gaussian elimination with partial pivoting
(pivot row selected/extracted with indicator arithmetic -- no data
dependent control flow, identical instruction stream for all walkers).
Engine split: DVE does all 2-input elementwise + reductions, ScalarE all
transcendentals/1-input chains (they overlap cleanly), GpSimd only tiny
memsets (its big ops starve DVE through SBUF port contention).
"""

import os
import sys
import numpy as np
from contextlib import ExitStack

for _p in ("/opt/trn_rl_repo", "/opt/pypackages"):
    if _p not in sys.path:
        sys.path.append(_p)

import concourse.bass as bass
import concourse.bacc as bacc
import concourse.mybir as mybir
import concourse.tile as tile
from concourse.bass import AP
from concourse.bass_utils import run_bass_kernel_spmd

P = 128          # partitions = walkers per core
NCORES = 8
B = 1024
N = 64           # electrons per walker
NS = 32          # electrons / orbitals per spin
L = 20.0
F32 = mybir.dt.float32
AF = mybir.ActivationFunctionType
OP = mybir.AluOpType
AX = mybir.AxisListType


def _centers():
    n = 1
    while n ** 3 < NS:
        n += 1
    a = L / n
    coords = np.linspace(0.0, L - a, n)
    grid = np.stack(np.meshgrid(coords, coords, coords, indexing="ij"), axis=-1)
    grid = grid.reshape(-1, 3)
    cu = grid[:NS].astype(np.float32)
    cd = (grid + a / 2)[:NS].astype(np.float32)
    return cu, cd


def _jastrow_consts():
    dens = np.float32(N / L ** 3)
    A = np.float32(1.0) / np.sqrt(np.float32(4 * np.pi) * dens, dtype=np.float32)
    Fs = np.sqrt(np.float32(2.0) * A, dtype=np.float32)
    Fd = np.sqrt(A, dtype=np.float32)
    return float(A), float(Fs), float(Fd)


def _build(alpha: float) -> bass.Bass:
    nc = bacc.Bacc()
    xsh = nc.declare_dram_parameter("xsh", [P, 3, N], F32, isOutput=False)
    cst = nc.declare_dram_parameter("cst", [P, 3, 2, NS], F32, isOutput=False)
    wcs = nc.declare_dram_parameter("wcs", [P, 2, NS], F32, isOutput=False)
    outp = nc.declare_dram_parameter("out", [P, 1], F32, isOutput=True)

    aL2 = float(alpha * L * L)
    s2aL = float(2.0 * alpha * L)
    Aj, Fs, Fd = _jastrow_consts()
    WMIN = float(1.0 - (1.0 - 1e-5) ** 2)   # lower clamp of w = 1 - x^2

    with ExitStack() as ctx:
        tc = ctx.enter_context(tile.TileContext(nc))
        pool = ctx.enter_context(tc.tile_pool(name="main", bufs=1))

        # ---- loads & small constants ----
        xe = pool.tile([P, 3, N], F32, tag="xe")
        nc.default_dma_engine.dma_start(xe, xsh[:])
        ce = pool.tile([P, 3, 2, NS], F32, tag="ce")
        nc.default_dma_engine.dma_start(ce, cst[:])

        biasc = pool.tile([P, 4], F32, tag="biasc")
        nc.gpsimd.memset(biasc[:, 0:1], -aL2)        # Exp image bias
        nc.gpsimd.memset(biasc[:, 1:2], -L / 2)      # Abs bias
        nc.gpsimd.memset(biasc[:, 2:3], L / 2)       # Square bias
        nc.gpsimd.memset(biasc[:, 3:4], 1e-37)       # Ln guard bias

        # =========================================================
        # Slater matrices, column-major: A[p, s, j, i] = Phi[i, j]
        #   f_axis = e0 * (1 + p+ + p-),   Phi = fx*fy*fz
        # =========================================================
        Abuf = pool.tile([P, 2, NS, NS], F32, tag="Abuf")
        prod = pool.tile([P, 2, NS, NS], F32, tag="prod")
        dbuf = pool.tile([P, 2, NS, NS], F32, tag="dbuf")
        t2 = pool.tile([P, 2, NS, NS], F32, tag="t2")
        t3 = pool.tile([P, 2, NS, NS], F32, tag="t3")
        t4 = pool.tile([P, 2, NS, NS], F32, tag="t4")
        t5 = pool.tile([P, 2, NS, NS], F32, tag="t5")

        for c in range(3):
            # d[p, s, j, i] = x_i - c_j
            xi = xe[:, c, :].rearrange("p (s i) -> p s i", s=2)
            xi = xi[:, :, None, :].broadcast_to([P, 2, NS, NS])
            cj = ce[:, c][:, :, :, None].broadcast_to([P, 2, NS, NS])
            nc.vector.tensor_tensor(dbuf, xi, cj, OP.subtract)
            nc.scalar.activation(t2, dbuf, AF.Square)                        # d^2
            nc.scalar.activation(t3, t2, AF.Exp, scale=-alpha)               # e0
            nc.scalar.activation(t4, dbuf, AF.Exp,
                                 bias=biasc[:, 0:1], scale=-s2aL)            # p+
            nc.scalar.activation(t5, dbuf, AF.Exp,
                                 bias=biasc[:, 0:1], scale=s2aL)             # p-
            nc.vector.tensor_tensor(t4, t4, t5, OP.add)                      # q
            dst = prod if c == 0 else t5
            # f = (q + 1) * e0
            nc.vector.scalar_tensor_tensor(dst, t4, 1.0, t3, OP.add, OP.mult)
            if c == 1:
                nc.vector.tensor_tensor(prod, prod, t5, OP.mult)
            elif c == 2:
                nc.vector.tensor_tensor(Abuf, prod, t5, OP.mult)

        # =========================================================
        # Jastrow front: pairwise wrapped r^2 (DVE dx + ScalarE chain)
        # =========================================================
        jacc = pool.tile([P, N, N], F32, tag="jacc")
        jt1 = pool.tile([P, N, N], F32, tag="jt1")
        jt2 = pool.tile([P, N, N], F32, tag="jt2")
        jt3 = pool.tile([P, N, N], F32, tag="jt3")
        jt4 = pool.tile([P, N, N], F32, tag="jt4")

        for c in range(3):
            xc = xe[:, c, :]
            nc.vector.tensor_tensor(
                jt1,
                xc[:, :, None].broadcast_to([P, N, N]),
                xc[:, None, :].broadcast_to([P, N, N]),
                OP.subtract,
            )                                                               # dx
            nc.scalar.activation(jt2, jt1, AF.Abs)                          # u=|dx|
            nc.scalar.activation(jt1, jt2, AF.Abs, bias=biasc[:, 1:2])      # b=|u-10|
            dst = jacc if c == 0 else jt2
            # wrapped squared axis distance: (10 - b)^2
            nc.scalar.activation(dst, jt1, AF.Square,
                                 bias=biasc[:, 2:3], scale=-1.0)
            if c > 0:
                nc.vector.tensor_tensor(jacc, jacc, jt2, OP.add)

        # kill the diagonal: r2 := 1000 => decay underflows to exactly 0
        diag = AP(jacc.tensor, jacc.offset, [list(jacc.ap[0]), [N + 1, N]])
        nc.gpsimd.memset(diag, 1000.0)

        # w = max(1 - r2/100, WMIN)  (= 1 - x^2 with the reference clip)
        nc.vector.tensor_scalar(jt4, jacc, -0.01, 1.0, OP.mult, OP.add)
        nc.vector.tensor_scalar(jt4, jt4, WMIN, None, OP.max)

        # ScalarE-only chains (overlap the GE below):
        nc.scalar.activation(jt1, jacc, AF.Ln)                              # ln r2
        nc.scalar.activation(jt2, jt1, AF.Exp, scale=-0.5)                  # q=1/r
        nc.scalar.activation(jacc, jt1, AF.Exp, scale=0.5)                  # r
        for (a0, b0, Fv) in ((0, 0, Fs), (NS, NS, Fs), (0, NS, Fd), (NS, 0, Fd)):
            nc.scalar.activation(
                jt3[:, a0:a0 + NS, b0:b0 + NS],
                jacc[:, a0:a0 + NS, b0:b0 + NS],
                AF.Exp,
                scale=-1.0 / Fv,
            )                                                               # e
        nc.scalar.activation(jt1, jt4, AF.Ln)                               # ln w
        nc.scalar.activation(jt4, jt1, AF.Exp, scale=-1.0)                  # 1/w
        nc.scalar.activation(jacc, jt4, AF.Exp, bias=1.0, scale=-1.0)       # decay

        # =========================================================
        # Batched Gaussian elimination w/ partial pivoting (both spins)
        # column-major A; pivot search on squared candidates
        # =========================================================
        maskw = pool.tile([P, 2, NS], F32, tag="maskw")
        c2b = pool.tile([P, 2, NS], F32, tag="c2b")
        c2m = pool.tile([P, 2, NS], F32, tag="c2m")
        indb = pool.tile([P, 2, NS], F32, tag="indb")
        Mb = pool.tile([P, 2], F32, tag="Mb")
        rpv = pool.tile([P, 2, 1], F32, tag="rpv")
        prowall = pool.tile([P, 2, NS, NS], F32, tag="prowall")
        scr = pool.tile([P, 2, NS, NS], F32, tag="scr")
        # maskw holds the tie-break weights; used rows go negative (-2 trick)
        nc.default_dma_engine.dma_start(maskw, wcs[:])

        def pivot_search(k):
            """c2/ind/extract/recip for step k (writes prowall[:, :, k], rpv)."""
            T = NS - k
            colk = Abuf[:, :, k, :]
            nc.vector.tensor_tensor(c2b, colk, colk, OP.mult)
            nc.vector.tensor_tensor(c2m, c2b, maskw, OP.mult)
            nc.vector.reduce_max(Mb, c2m, axis=AX.X)
            nc.vector.tensor_tensor(
                indb, c2m, Mb[:, :, None].broadcast_to([P, 2, NS]), OP.is_equal
            )
            for sp in (0, 1):
                nc.vector.tensor_tensor(
                    scr[:, sp, :T, :],
                    Abuf[:, sp, k:, :],
                    indb[:, sp, None, :].broadcast_to([P, T, NS]),
                    OP.mult,
                )
            for sp in (0, 1):
                nc.vector.reduce_sum(
                    prowall[:, sp, k, :T], scr[:, sp, :T, :], axis=AX.X
                )
            if k < NS - 1:
                nc.vector.reciprocal(rpv, prowall[:, :, k, 0:1])

        pivot_search(0)
        for k in range(NS - 1):
            # --- apply step-k elimination; pivot row self-annihilates,
            #     used rows are dead storage (negative maskw) ---
            T = NS - k
            for sp in (0, 1):
                # tU[j', i] = (colk_i * (1/pval)) * prow_j'   (j' = k+1..)
                nc.vector.scalar_tensor_tensor(
                    scr[:, sp, :T - 1, :],
                    Abuf[:, sp, k, None, :].broadcast_to([P, T - 1, NS]),
                    rpv[:, sp, :],
                    prowall[:, sp, k, 1:T, None].broadcast_to([P, T - 1, NS]),
                    OP.mult, OP.mult,
                )
            for sp in (0, 1):
                # column k+1 first: unblocks the next pivot search
                nc.vector.tensor_tensor(
                    Abuf[:, sp, k + 1, :], Abuf[:, sp, k + 1, :],
                    scr[:, sp, 0, :], OP.subtract
                )
            # retire pivot k from the candidate mask (spacer op)
            nc.vector.scalar_tensor_tensor(
                maskw, indb, -2.0, maskw, OP.mult, OP.add
            )
            # next step's pivot search on column k+1
            colk1 = Abuf[:, :, k + 1, :]
            nc.vector.tensor_tensor(c2b, colk1, colk1, OP.mult)
            nc.vector.tensor_tensor(c2m, c2b, maskw, OP.mult)
            nc.vector.reduce_max(Mb, c2m, axis=AX.X)
            nc.vector.tensor_tensor(
                indb, c2m, Mb[:, :, None].broadcast_to([P, 2, NS]), OP.is_equal
            )
            # rest of the step-k update
            if T > 2:
                for sp in (0, 1):
                    nc.vector.tensor_tensor(
                        Abuf[:, sp, k + 2:, :], Abuf[:, sp, k + 2:, :],
                        scr[:, sp, 1:T - 1, :], OP.subtract
                    )
            # extraction for step k+1
            T1 = T - 1
            for sp in (0, 1):
                nc.vector.tensor_tensor(
                    scr[:, sp, :T1, :],
                    Abuf[:, sp, k + 1:, :],
                    indb[:, sp, None, :].broadcast_to([P, T1, NS]),
                    OP.mult,
                )
            for sp in (0, 1):
                nc.vector.reduce_sum(
                    prowall[:, sp, k + 1, :T1], scr[:, sp, :T1, :], axis=AX.X
                )
            if k + 1 < NS - 1:
                nc.vector.reciprocal(rpv, prowall[:, :, k + 1, 0:1])

        # =========================================================
        # Jastrow tail + logdet reduction + combine
        # =========================================================
        nc.scalar.activation(jt1, jt3, AF.Copy, bias=1.0, scale=-1.0)      # 1-e
        nc.vector.tensor_tensor(jt1, jt1, jacc, OP.mult)                    # *decay
        nc.vector.tensor_tensor(jt1, jt1, jt2, OP.mult)                     # *(1/r)
        jsum = pool.tile([P, 1], F32, tag="jsum")
        nc.scalar.activation(jt3, jt1, AF.Copy, scale=-0.5 * Aj,
                             accum_out=jsum)

        labs = pool.tile([P, 2, NS], F32, tag="labs")
        lgb = pool.tile([P, 2, NS], F32, tag="lgb")
        nc.scalar.activation(labs, prowall[:, :, :, 0], AF.Abs)
        nc.scalar.activation(lgb, labs, AF.Ln, bias=biasc[:, 3:4])
        ld2 = pool.tile([P, 2], F32, tag="ld2")
        nc.vector.reduce_sum(ld2, lgb, axis=AX.X)
        ld1 = pool.tile([P, 1], F32, tag="ld1")
        nc.vector.reduce_sum(ld1, ld2, axis=AX.X)
        ob = pool.tile([P, 1], F32, tag="ob")
        nc.vector.tensor_tensor(ob, ld1, jsum, OP.add)
        nc.default_dma_engine.dma_start(outp[:], ob)

    nc.finalize()
    return nc


_CACHE = {}


def _get_built(alpha: float):
    key = round(alpha, 9)
    if key not in _CACHE:
        _CACHE[key] = _build(alpha)
    return _CACHE[key]


def _make_inputs(walkerRs: np.ndarray):
    cu, cd = _centers()
    cen = np.stack([cu, cd], 0)                   # (2, NS, 3)
    cst = np.ascontiguousarray(
        np.broadcast_to(cen.transpose(2, 0, 1)[None], (P, 3, 2, NS))
    ).astype(np.float32)
    w = (1.0 + np.arange(NS) * 2.0 ** -21).astype(np.float32)
    wcs = np.ascontiguousarray(
        np.broadcast_to(w[None, None, :], (P, 2, NS))
    ).astype(np.float32)
    in_maps = []
    for c in range(NCORES):
        sh = walkerRs[c * P:(c + 1) * P]          # (P, N, 3)
        xsh = np.ascontiguousarray(sh.transpose(0, 2, 1)).astype(np.float32)
        in_maps.append({"xsh": xsh, "cst": cst, "wcs": wcs})
    return in_maps


def kernel(walkerRs: np.ndarray, log_alpha: np.ndarray, _trace=False):
    walkerRs = np.asarray(walkerRs, dtype=np.float32)
    la = float(np.asarray(log_alpha))
    alpha = float(np.clip(np.exp(la), 55.0 / L ** 2, 300.0 / L ** 2))
    nc = _get_built(alpha)
    in_maps = _make_inputs(walkerRs)
    res = run_bass_kernel_spmd(nc, in_maps, list(range(NCORES)), trace=_trace)
    out = np.concatenate([res.results[i]["out"][:, 0] for i in range(NCORES)])
    if _trace:
        return out.astype(np.float32), res
    return out.astype(np.float32)
